# revision 1
# baseline (speedup 1.0000x reference)
"""ContrastiveLoss Trainium2 kernel.

Contract: kernel(feature, label) -> (loss, mean_pos, mean_neg), matching
reference.reference(). Full inputs in, full outputs out; internally sharded
across 8 NeuronCores.

Strategy: rows sorted by label on host (label-1 block first, n1 rows), so the
label-weighted exp row sums come free as range-split accum_out from the
scalar-engine exp pass. Each core receives the full sorted feature (columns)
plus its own 1024-row slice (rows) -> one uniform SPMD program, no
collectives; host sums the per-core scalar partials.
"""

import sys

sys.path.insert(0, "/opt/trn_rl_repo")

import numpy as np

import concourse.bass as bass
import concourse.mybir as mybir
import concourse.tile as tile
from concourse import bacc
from concourse.bass import ds, ts
from concourse.bass_utils import run_bass_kernel_spmd
from concourse.masks import make_identity

B = 8192
D = 128
N_CORES = 8
LOCAL = B // N_CORES          # 1024 rows per core
TILES_I = LOCAL // 128        # 8 local row tiles per core
GROUPS = B // LOCAL           # 8 fsort groups of 8 tiles
CHUNK = 2048                  # psum chunk width (4 banks)
NCHUNK = B // CHUNK
SUB = 512                     # matmul moving width & zTrc tile width
NSUB = B // SUB               # 16 column tiles
EPS = 1e-8

FP32 = mybir.dt.float32
F32R = mybir.dt.float32r
AF = mybir.ActivationFunctionType
ALU = mybir.AluOpType
AX = mybir.AxisListType


def _build_kernel(n1: int):
    nc = bacc.Bacc("TRN2", target_bir_lowering=False, debug=False,
                   num_devices=N_CORES)
    fsort = nc.dram_tensor("fsort", [B, D], FP32, kind="ExternalInput").ap()
    floc = nc.dram_tensor("floc", [LOCAL, D], FP32, kind="ExternalInput").ap()
    labloc = nc.dram_tensor("labloc", [128, TILES_I], FP32,
                            kind="ExternalInput").ap()
    outp = nc.dram_tensor("outp", [128, 3 * TILES_I], FP32,
                          kind="ExternalOutput").ap()

    # activation segments per row tile: (lo, hi, is_label1), label-1 first
    segs = []
    for q in range(NCHUNK):
        lo, hi = q * CHUNK, (q + 1) * CHUNK
        if n1 <= lo:
            segs.append((lo, hi, False))
        elif n1 >= hi:
            segs.append((lo, hi, True))
        else:
            segs.append((lo, n1, True))
            segs.append((n1, hi, False))
    segs = [s for s in segs if s[2]] + [s for s in segs if not s[2]]
    nseg = len(segs)
    cb = sum(1 for s in segs if s[2])

    with tile.TileContext(nc) as tc:
        with (
            tc.tile_pool(name="zc", bufs=1) as zc,
            tc.tile_pool(name="fgp", bufs=1) as fgp,
            tc.tile_pool(name="deadp", bufs=2) as deadp,
            tc.tile_pool(name="small", bufs=1) as small,
            tc.tile_pool(name="psum", bufs=2, space=bass.MemorySpace.PSUM) as psum,
        ):
            ztc = [zc.tile([128, SUB], F32R, tag=f"ztc{k}", name=f"ztc{k}")
                   for k in range(NSUB)]
            zlT = zc.tile([128, LOCAL], F32R, tag="zlT")
            ident = small.tile([128, 128], FP32, tag="ident")
            make_identity(nc, ident[:])

            lab_loc = small.tile([128, TILES_I], FP32, tag="labloc")
            nc.gpsimd.dma_start(lab_loc[:], labloc)

            nrm2l = small.tile([128, TILES_I], FP32, tag="nrm2l")
            scll = small.tile([128, TILES_I], FP32, tag="scll")

            def norm_group(src3, fg, nrm2c, sclc, dma_eng):
                """Load one 8-tile group, compute row norms, scale rows."""
                dma_eng.dma_start(fg[:], src3)
                for t in range(TILES_I):
                    dead = deadp.tile([128, 128], FP32, tag="dead")
                    nc.vector.scalar_tensor_tensor(
                        out=dead[:], in0=fg[:, t, :], scalar=1.0,
                        in1=fg[:, t, :], op0=ALU.mult, op1=ALU.mult,
                        accum_out=nrm2c[:, t:t + 1])
                # scl = exp(-0.5*ln(max(nrm2,1e-24))) = 1/max(||f||,1e-12)
                nc.vector.tensor_scalar_max(out=sclc[:], in0=nrm2c[:],
                                            scalar1=1e-24)
                nc.scalar.activation(sclc[:], sclc[:], AF.Ln)
                nc.scalar.activation(sclc[:], sclc[:], AF.Exp, scale=-0.5)
                for t in range(TILES_I):
                    nc.gpsimd.tensor_scalar_mul(
                        out=fg[:, t, :], in0=fg[:, t, :],
                        scalar1=sclc[:, t:t + 1])

            def transpose_group(fg, dst):
                """dst[h] <- transpose of fg tiles 4h..4h+3 ([128,512] each)."""
                for h in range(2):
                    pt = psum.tile([128, CHUNK], FP32, tag="ps")
                    for k in range(4):
                        nc.tensor.transpose(pt[:, ts(k, 128)],
                                            fg[:, h * 4 + k, :], ident[:])
                    nc.vector.tensor_copy(dst[h], pt[:, 0:SUB])

            # local rows first (zlT needed by every main matmul)
            fl = fgp.tile([128, TILES_I, D], FP32, tag="fl")
            norm_group(floc.rearrange("(t p) d -> p t d", p=128), fl,
                       nrm2l, scll, nc.gpsimd)
            transpose_group(fl, [zlT[:, 0:SUB], zlT[:, SUB:2 * SUB]])

            fs4 = fsort.rearrange("(g t p) d -> g p t d", p=128, t=TILES_I)
            nrm2a = small.tile([128, GROUPS * TILES_I], FP32, tag="nrm2a")
            scla = small.tile([128, GROUPS * TILES_I], FP32, tag="scla")
            for g in range(GROUPS):
                fg = fgp.tile([128, TILES_I, D], FP32, tag=f"fg{g}")
                sl = ds(g * TILES_I, TILES_I)
                norm_group(fs4[g], fg, nrm2a[:, sl], scla[:, sl],
                           nc.sync if g % 2 == 0 else nc.gpsimd)
                transpose_group(fg, [ztc[2 * g][:], ztc[2 * g + 1][:]])

            # S = sum_j z_j ; S1 = sum_{j<n1} z_j  (sorted: 1s first)
            scols = small.tile([128, NSUB], FP32, tag="scols")
            for k in range(NSUB):
                nc.vector.reduce_sum(scols[:, k:k + 1], ztc[k][:], axis=AX.X)
            svec = small.tile([128, 2], FP32, tag="svec")
            s1p = small.tile([128, 2], FP32, tag="s1p")
            nc.vector.reduce_sum(svec[:, 0:1], scols[:], axis=AX.X)
            kf, w1 = n1 // SUB, n1 % SUB
            if kf > 0:
                nc.vector.reduce_sum(s1p[:, 0:1], scols[:, 0:kf], axis=AX.X)
            else:
                nc.vector.memset(s1p[:, 0:1], 0.0)
            if w1 > 0:
                nc.vector.reduce_sum(s1p[:, 1:2], ztc[kf][:, 0:w1], axis=AX.X)
            else:
                nc.vector.memset(s1p[:, 1:2], 0.0)
            nc.vector.reduce_sum(svec[:, 1:2], s1p[:], axis=AX.X)
            svecr = small.tile([128, 2], F32R, tag="svecr")
            nc.vector.tensor_copy(svecr[:], svec[:])

            # per-row-tile sim row sums: [z_i . S, z_i . S1]
            tall = small.tile([128, TILES_I], FP32, tag="tall")
            t1 = small.tile([128, TILES_I], FP32, tag="t1")
            for t in range(TILES_I):
                tp = psum.tile([128, CHUNK], FP32, tag="ps")
                nc.tensor.matmul(tp[:, 0:2], lhsT=zlT[:, ts(t, 128)],
                                 rhs=svecr[:], start=True, stop=True)
                nc.vector.tensor_copy(tall[:, t:t + 1], tp[:, 0:1])
                nc.vector.tensor_copy(t1[:, t:t + 1], tp[:, 1:2])

            # ---- main loop, chunk-major for setup/main overlap ----
            sacc = small.tile([128, TILES_I, nseg], FP32, tag="sacc")
            for q in range(NCHUNK):
                for t in range(TILES_I):
                    ps = psum.tile([128, CHUNK], FP32, tag="ps")
                    for s in range(CHUNK // SUB):
                        nc.tensor.matmul(
                            ps[:, ts(s, SUB)], lhsT=zlT[:, ts(t, 128)],
                            rhs=ztc[q * (CHUNK // SUB) + s][:],
                            start=True, stop=True)
                    for si, (lo, hi, _one) in enumerate(segs):
                        if lo >= q * CHUNK and hi <= (q + 1) * CHUNK:
                            nc.scalar.activation(
                                ps[:, lo - q * CHUNK:hi - q * CHUNK],
                                ps[:, lo - q * CHUNK:hi - q * CHUNK],
                                AF.Exp, scale=2.0,
                                accum_out=sacc[:, t, si:si + 1])

            # ---- finals ----
            simii = small.tile([128, TILES_I], FP32, tag="simii")
            eii = small.tile([128, TILES_I], FP32, tag="eii")
            nc.vector.tensor_mul(simii[:], nrm2l[:], scll[:])
            nc.vector.tensor_mul(simii[:], simii[:], scll[:])
            nc.scalar.activation(eii[:], simii[:], AF.Exp, scale=2.0)

            s1r = small.tile([128, TILES_I], FP32, tag="s1r")
            s0r = small.tile([128, TILES_I], FP32, tag="s0r")
            if cb > 0:
                nc.vector.reduce_sum(s1r[:], sacc[:, :, 0:cb], axis=AX.X)
            else:
                nc.vector.memset(s1r[:], 0.0)
            if cb < nseg:
                nc.vector.reduce_sum(s0r[:], sacc[:, :, cb:nseg], axis=AX.X)
            else:
                nc.vector.memset(s0r[:], 0.0)

            fin = small.tile([128, TILES_I], FP32, tag="fin")
            outs = small.tile([128, 3 * TILES_I], FP32, tag="outs")
            sall = small.tile([128, TILES_I], FP32, tag="sall")
            nc.vector.tensor_add(sall[:], s1r[:], s0r[:])
            # same = s0 + lab*(s1-s0)
            nc.vector.tensor_sub(fin[:], s1r[:], s0r[:])
            nc.vector.tensor_mul(fin[:], fin[:], lab_loc[:])
            nc.vector.tensor_add(fin[:], fin[:], s0r[:])
            num = small.tile([128, TILES_I], FP32, tag="num")
            nc.vector.tensor_sub(num[:], fin[:], eii[:])
            dennum = small.tile([128, TILES_I], FP32, tag="dennum")
            nc.vector.tensor_sub(dennum[:], sall[:], eii[:])
            nc.vector.tensor_scalar_add(out=num[:], in0=num[:], scalar1=EPS)
            # loss_row = ln(den+num) - ln(num+eps)
            lg1 = small.tile([128, TILES_I], FP32, tag="lg1")
            nc.scalar.activation(lg1[:], dennum[:], AF.Ln)
            nc.scalar.activation(outs[:, 0:TILES_I], num[:], AF.Ln)
            nc.vector.tensor_sub(outs[:, 0:TILES_I], lg1[:], outs[:, 0:TILES_I])

            # same_t = u + lab*(t1-u), u = tall - t1
            u = small.tile([128, TILES_I], FP32, tag="u")
            nc.vector.tensor_sub(u[:], tall[:], t1[:])
            nc.vector.tensor_sub(fin[:], t1[:], u[:])
            nc.vector.tensor_mul(fin[:], fin[:], lab_loc[:])
            nc.vector.tensor_add(fin[:], fin[:], u[:])
            nc.vector.tensor_sub(outs[:, TILES_I:2 * TILES_I], fin[:], simii[:])
            nc.vector.tensor_sub(outs[:, 2 * TILES_I:3 * TILES_I], tall[:], fin[:])

            nc.sync.dma_start(outp, outs[:])

    nc.compile()
    return nc


_NC_CACHE = {}


def _get_nc(n1: int = 4083):
    if n1 not in _NC_CACHE:
        _NC_CACHE[n1] = _build_kernel(n1)
    return _NC_CACHE[n1]


def prepare(feature: np.ndarray, label: np.ndarray):
    """Sort rows by label (1s first); build per-core input maps."""
    feature = np.ascontiguousarray(feature, dtype=np.float32)
    lab = np.asarray(label)
    perm = np.argsort(-lab, kind="stable")
    n1 = int((lab == 1).sum())
    fsort = np.ascontiguousarray(feature[perm])
    lsort = lab[perm].astype(np.float32)
    in_maps = []
    for c in range(N_CORES):
        sl = slice(c * LOCAL, (c + 1) * LOCAL)
        in_maps.append({
            "fsort": fsort,
            "floc": np.ascontiguousarray(fsort[sl]),
            "labloc": np.ascontiguousarray(
                lsort[sl].reshape(TILES_I, 128).T),
        })
    return n1, in_maps


def combine(results):
    P = np.stack([np.asarray(r["outp"], dtype=np.float64) for r in results])
    loss = P[:, :, 0:TILES_I].sum() / B
    mean_pos = P[:, :, TILES_I:2 * TILES_I].sum() / (float(B) * B)
    mean_neg = P[:, :, 2 * TILES_I:3 * TILES_I].sum() / (float(B) * B)
    return (np.float32(loss), np.float32(mean_pos), np.float32(mean_neg))


def run_on_hw(feature, label, **kwargs):
    n1, in_maps = prepare(feature, label)
    nc = _get_nc(n1)
    res = run_bass_kernel_spmd(nc, in_maps,
                               core_ids=list(range(N_CORES)), **kwargs)
    return combine(res.results), res


def kernel(feature: np.ndarray, label: np.ndarray):
    out, _ = run_on_hw(feature, label)
    return out



# revision 3
# speedup vs baseline: 3.1807x; 3.1807x over previous
"""ContrastiveLoss Trainium2 kernel.

Contract: kernel(feature, label) -> (loss, mean_pos, mean_neg), matching
reference.reference(). Full inputs in, full outputs out; internally sharded
across 8 NeuronCores.

Strategy: host sorts rows by label (1s first) and L2-normalizes -> z.
Device work is the O(B^2) part only: each core holds z^T (columns) and its
own 1024-row slice z_loc^T (matmul weights), computes its [1024, 8192]
similarity slab in PSUM chunks, applies exp(2*sim) on the scalar engine with
accum_out row sums split at the label boundary n1 (columns are label-sorted,
so the masked sums are contiguous-range sums). Output is the per-(tile,
segment) accumulator array; host combines partials, applies log, and gets
mean_pos / mean_neg in closed form from S1/S0 (sum of z over each label).
"""

import sys

sys.path.insert(0, "/opt/trn_rl_repo")

import ml_dtypes
import numpy as np

import concourse.bass as bass
import concourse.mybir as mybir
import concourse.tile as tile
from concourse import bacc
from concourse.bass import ds, ts
from concourse.bass_utils import run_bass_kernel_spmd

B = 8192
D = 128
N_CORES = 8
LOCAL = B // N_CORES          # 1024 rows per core
TILES_I = LOCAL // 128        # 8 local row tiles per core
CHUNK = 2048                  # psum chunk width (4 banks)
NCHUNK = B // CHUNK
SUB = 512                     # matmul moving width & zT column tile width
NSUB = B // SUB               # 16 column tiles
EPS = 1e-8
TEMPERATURE = 0.5

FP32 = mybir.dt.float32
BF16 = mybir.dt.bfloat16
AF = mybir.ActivationFunctionType


def _segments(n1: int):
    """Per-chunk activation column ranges split at the label boundary.

    Returns (segs, cb): segs is a list of (lo, hi) with label-1 columns
    first, cb = number of label-1 segments.
    """
    one, zero = [], []
    for q in range(NCHUNK):
        lo, hi = q * CHUNK, (q + 1) * CHUNK
        if n1 <= lo:
            zero.append((lo, hi))
        elif n1 >= hi:
            one.append((lo, hi))
        else:
            if n1 > lo:
                one.append((lo, n1))
            if hi > n1:
                zero.append((n1, hi))
    return one + zero, len(one)


def _build_kernel(n1: int):
    nc = bacc.Bacc("TRN2", target_bir_lowering=False, debug=False,
                   num_devices=N_CORES)
    zt = nc.dram_tensor("zt", [D, B], BF16, kind="ExternalInput").ap()
    zlt = nc.dram_tensor("zlt", [D, LOCAL], BF16, kind="ExternalInput").ap()
    segs, cb = _segments(n1)
    nseg = len(segs)
    outp = nc.dram_tensor("outp", [128, TILES_I * nseg], FP32,
                          kind="ExternalOutput").ap()

    with tile.TileContext(nc) as tc:
        with (
            tc.tile_pool(name="zc", bufs=1) as zc,
            tc.tile_pool(name="small", bufs=1) as small,
            tc.tile_pool(name="psum", bufs=2, space=bass.MemorySpace.PSUM) as psum,
        ):
            # column tiles of z^T and the local weights, DMA'd straight in
            ztc = [zc.tile([128, SUB], BF16, tag=f"ztc{k}", name=f"ztc{k}")
                   for k in range(NSUB)]
            zlT = zc.tile([128, LOCAL], BF16, tag="zlT")
            nc.sync.dma_start(zlT[:], zlt)
            for k in range(NSUB):
                eng = nc.sync if k % 2 == 0 else nc.gpsimd
                eng.dma_start(ztc[k][:], zt[:, ds(k * SUB, SUB)])

            sacc = small.tile([128, TILES_I, nseg], FP32, tag="sacc")
            for q in range(NCHUNK):
                for t in range(TILES_I):
                    ps = psum.tile([128, CHUNK], FP32, tag="ps")
                    for s in range(CHUNK // SUB):
                        nc.tensor.matmul(
                            ps[:, ts(s, SUB)], lhsT=zlT[:, ts(t, 128)],
                            rhs=ztc[q * (CHUNK // SUB) + s][:],
                            start=True, stop=True)
                    for si, (lo, hi) in enumerate(segs):
                        if lo >= q * CHUNK and hi <= (q + 1) * CHUNK:
                            nc.scalar.activation(
                                ps[:, lo - q * CHUNK:hi - q * CHUNK],
                                ps[:, lo - q * CHUNK:hi - q * CHUNK],
                                AF.Exp, scale=2.0,
                                accum_out=sacc[:, t, si:si + 1])

            nc.sync.dma_start(outp, sacc[:].rearrange("p t s -> p (t s)"))

    nc.compile()
    return nc, nseg, cb


_NC_CACHE = {}


def _get_nc(n1: int = 4083):
    if n1 not in _NC_CACHE:
        _NC_CACHE[n1] = _build_kernel(n1)
    return _NC_CACHE[n1]


def prepare(feature: np.ndarray, label: np.ndarray):
    """Sort rows by label (1s first), L2-normalize; per-core input maps."""
    feature = np.ascontiguousarray(feature, dtype=np.float32)
    lab = np.asarray(label)
    perm = np.argsort(-lab, kind="stable")
    n1 = int((lab == 1).sum())
    fsort = feature[perm]
    nrm = np.sqrt((fsort.astype(np.float64) ** 2).sum(1))
    z = (fsort / np.maximum(nrm, 1e-12)[:, None].astype(np.float32)).astype(
        np.float32)
    zT = np.ascontiguousarray(z.T.astype(ml_dtypes.bfloat16))
    in_maps = []
    for c in range(N_CORES):
        in_maps.append({
            "zt": zT,
            "zlt": np.ascontiguousarray(zT[:, c * LOCAL:(c + 1) * LOCAL]),
        })
    lsort = lab[perm].astype(np.float64)
    return n1, in_maps, z, lsort


def combine(results, nseg, cb, z, lsort):
    """Host-side finals from per-core accumulator partials (float64)."""
    # P[c, p, t, s]: rows of core c tile t are sorted rows c*1024+t*128+p
    P = np.stack([np.asarray(r["outp"], dtype=np.float64).reshape(
        128, TILES_I, nseg) for r in results])
    s1 = P[:, :, :, :cb].sum(3)          # [core, p, t]
    s0 = P[:, :, :, cb:].sum(3)
    # -> per sorted row
    s1 = s1.transpose(0, 2, 1).reshape(B)
    s0 = s0.transpose(0, 2, 1).reshape(B)
    sall = s1 + s0
    eii = np.exp(2.0)                    # z_i . z_i == 1 exactly
    same = np.where(lsort == 1.0, s1, s0)
    num = same - eii
    dennum = sall - eii
    loss = float(np.sum(np.log(dennum) - np.log(num + EPS)) / B)

    zd = z.astype(np.float64)
    S1 = (zd * lsort[:, None]).sum(0)
    S0 = zd.sum(0) - S1
    mean_pos = (S1 @ S1 + S0 @ S0 - B) / (float(B) * B)
    mean_neg = (2.0 * (S1 @ S0)) / (float(B) * B)
    return (np.float32(loss), np.float32(mean_pos), np.float32(mean_neg))


def run_on_hw(feature, label, **kwargs):
    n1, in_maps, z, lsort = prepare(feature, label)
    nc, nseg, cb = _get_nc(n1)
    res = run_bass_kernel_spmd(nc, in_maps,
                               core_ids=list(range(N_CORES)), **kwargs)
    return combine(res.results, nseg, cb, z, lsort), res


def kernel(feature: np.ndarray, label: np.ndarray):
    out, _ = run_on_hw(feature, label)
    return out


# revision 11
# speedup vs baseline: 4.1300x; 1.2985x over previous
"""ContrastiveLoss Trainium2 kernel (symmetric / triangle scheme).

Contract: kernel(feature, label) -> (loss, mean_pos, mean_neg), matching
reference.reference(). Full inputs in, full outputs out; internally sharded
across 8 NeuronCores.

Strategy: host sorts rows by label (1s first) and L2-normalizes -> z. The
sim matrix exp(2 * z z^T) is symmetric, so each unordered pair is computed
once: global row tile T (of 64 x 128 rows) computes columns [128T, 8192).
Row sums over those columns come from the scalar-engine exp accum; the
transposed contributions (partners i < 128T for each row) are column sums,
computed by streaming the exp'd block (SBUF, bf16) through the PE against a
[label-indicator | ones] stationary matrix, accumulated in PSUM slots packed
by quadrant (16 column-tile slots x [2, 512] at partition offsets 0/32/64/96
across 4 banks). Tiles are paired (T, 63-T) so every core does exactly 260
column-tiles of work; per-core bodies live in a tc.Switch(partition_id, 8).
Host combines row-sum and column-sum partials (float64), applies the log,
and gets mean_pos / mean_neg in closed form from S1/S0.
"""

import sys

sys.path.insert(0, "/opt/trn_rl_repo")

import ml_dtypes
import numpy as np

import concourse.bass as bass
import concourse.mybir as mybir
import concourse.tile as tile
from concourse import bacc
from concourse.bass import ds, ts
from concourse.bass_utils import run_bass_kernel_spmd

B = 8192
D = 128
N_CORES = 8
NT = B // 128                 # 64 global row tiles
TILES_I = 8                   # row tiles per core
CHUNK = 1024                  # psum sim-chunk width (2 banks)
GT = 512                      # global column tile width (psum bank)
NGT = B // GT                 # 16 global column tiles
EPS = 1e-8

FP32 = mybir.dt.float32
BF16 = mybir.dt.bfloat16
AF = mybir.ActivationFunctionType
MAX_SLOTS = 10                # rowsum accum slots per row tile (padded)


def core_tiles(c):
    """Global row-tile indices owned by core c (pairs (T, 63-T))."""
    low = [4 * c + i for i in range(4)]
    high = [63 - t for t in low]
    return low + sorted(high)


def tile_chunks(T, n1):
    """Chunk/segment structure for global row tile T.

    Returns list of chunks; each chunk is a dict with:
      g0: first global column covered by the psum tile (gtile-aligned)
      a, b: computed column range (a >= 128T)
      segs: list of (lo, hi, is_label1) activation ranges (split at n1)
      cols: list of (gt, lo, hi) per-gtile matmul pieces
    """
    diag = 128 * T
    start = GT * (diag // GT)
    chunks = []
    g = start
    nslot = 0
    while g < B:
        a, b = max(diag, g), min(B, g + CHUNK)
        segs = []
        if a < n1 < b:
            segs.append((a, n1, True))
            segs.append((n1, b, False))
        else:
            segs.append((a, b, a < n1))
        cols = []
        for gt in range(g // GT, (b + GT - 1) // GT):
            lo, hi = max(a, gt * GT), min(b, (gt + 1) * GT)
            if lo < hi:
                cols.append((gt, lo, hi))
        chunks.append(dict(g0=g, a=a, b=b, segs=segs, cols=cols,
                           slot0=nslot))
        nslot += len(segs)
        g += CHUNK
    assert nslot <= MAX_SLOTS, (T, nslot)
    return chunks


def _build_kernel(n1: int):
    nc = bacc.Bacc("TRN2", target_bir_lowering=False, debug=False,
                   num_devices=N_CORES)
    zt = nc.dram_tensor("zt", [D, B], BF16, kind="ExternalInput").ap()
    zlt = nc.dram_tensor("zlt", [D, 128 * TILES_I], BF16,
                         kind="ExternalInput").ap()
    ind = nc.dram_tensor("ind", [128, 2 * TILES_I], BF16,
                         kind="ExternalInput").ap()
    outp = nc.dram_tensor("outp", [128, TILES_I * MAX_SLOTS], FP32,
                          kind="ExternalOutput").ap()
    # colsum slot banks: quadrant rows 32q:32q+2 of bank b hold gtile 4b+q
    outc = nc.dram_tensor("outc", [4, 128, GT], FP32,
                          kind="ExternalOutput").ap()

    with tile.TileContext(nc) as tc:
        with (
            tc.tile_pool(name="zc", bufs=1) as zc,
            tc.tile_pool(name="small", bufs=1) as small,
            tc.tile_pool(name="eb", bufs=2) as ebp,
            tc.tile_pool(name="psum", bufs=2, space=bass.MemorySpace.PSUM) as psum,
            tc.tile_pool(name="pcolp", bufs=1, space=bass.MemorySpace.PSUM) as pcolp,
        ):
            # ---- shared (uniform) preamble: DMAs + psum colsum init ----
            ztc = [zc.tile([128, GT], BF16, tag=f"ztc{k}", name=f"ztc{k}")
                   for k in range(NGT)]
            zlT = zc.tile([128, 128 * TILES_I], BF16, tag="zlT")
            indt = small.tile([128, 2 * TILES_I], BF16, tag="indt")
            sacc = small.tile([128, TILES_I, MAX_SLOTS], FP32, tag="sacc")
            nc.sync.dma_start(zlT[:], zlt)
            nc.gpsimd.dma_start(indt[:], ind)
            engs = [nc.sync, nc.gpsimd, nc.scalar]
            order = [0, 4, 8, 1, 5, 9, 2, 6, 10, 3, 7, 11, 12, 13, 14, 15]
            for i, k in enumerate(order):
                engs[i % 3].dma_start(ztc[k][:], zt[:, ds(k * GT, GT)])
            nc.vector.memset(sacc[:], 0.0)

            pcb = [pcolp.tile([128, GT], FP32, tag=f"pcb{b}", name=f"pcb{b}")
                   for b in range(4)]
            for b in range(4):
                nc.vector.memset(pcb[b][:], 0.0)

            def cslot(gt):
                return pcb[gt // 4], 32 * (gt % 4)

            # ---- per-core bodies ----
            pid = nc.partition_id()
            for c in tc.Switch(pid, N_CORES):
                tlist = core_tiles(c)
                # software-pipelined emission: colsums lag one chunk
                pending = None  # (ebuf_tile, chunk, T)
                for t, T in enumerate(tlist):
                    for ch in tile_chunks(T, n1):
                        g0, a, b = ch["g0"], ch["a"], ch["b"]
                        w = b - g0
                        ps = psum.tile([128, CHUNK], FP32, tag="ps")
                        for (gt, lo, hi) in ch["cols"]:
                            nc.tensor.matmul(
                                ps[:, lo - g0:hi - g0],
                                lhsT=zlT[:, ts(t, 128)],
                                rhs=ztc[gt][:, lo - gt * GT:hi - gt * GT],
                                start=True, stop=True)
                        if pending is not None:
                            _emit_colsums(nc, cslot, indt, *pending)
                        eb = ebp.tile([128, CHUNK], BF16, tag="eb")
                        for si, (lo, hi, _l1) in enumerate(ch["segs"]):
                            slot = ch["slot0"] + si
                            nc.scalar.activation(
                                eb[:, lo - g0:hi - g0],
                                ps[:, lo - g0:hi - g0],
                                AF.Exp, scale=2.0,
                                accum_out=sacc[:, t, slot:slot + 1])
                        pending = (eb, ch, t, T)
                    if pending is not None:
                        _emit_colsums(nc, cslot, indt, *pending)
                        pending = None

            # ---- shared epilogue: outputs ----
            nc.sync.dma_start(outp, sacc[:].rearrange("p t s -> p (t s)"))
            for b in range(4):
                cb_sb = small.tile([128, GT], FP32, tag=f"cbsb{b}",
                                   name=f"cbsb{b}")
                nc.vector.tensor_copy(cb_sb[:], pcb[b][:])
                nc.gpsimd.dma_start(outc[b], cb_sb[:])

    nc.compile()
    return nc


def _emit_colsums(nc, cslot, indt, eb, ch, t, T):
    """Column-sum matmuls for a finished chunk (exp'd values in eb)."""
    g0 = ch["g0"]
    cstart = 128 * (T + 1)   # exclude the diagonal tile
    for (gt, lo, hi) in ch["cols"]:
        lo = max(lo, cstart)
        if lo >= hi:
            continue
        pc, qoff = cslot(gt)
        nc.tensor.matmul(
            pc[qoff:qoff + 2, lo - gt * 512:hi - gt * 512],
            lhsT=indt[:, 2 * t:2 * t + 2],
            rhs=eb[:, lo - g0:hi - g0],
            start=False, stop=True, skip_group_check=True,
            tile_position=(0, qoff))


def plan_slots(n1):
    """Rowsum accumulator slot map; returns {T: [(slot, lo, hi, l1)]}."""
    plan = {}
    for T in range(NT):
        slots = []
        for ch in tile_chunks(T, n1):
            for si, (lo, hi, l1) in enumerate(ch["segs"]):
                slots.append((ch["slot0"] + si, lo, hi, l1))
        plan[T] = slots
    return plan


_NC_CACHE = {}


def _get_nc(n1: int = 4083):
    if n1 not in _NC_CACHE:
        _NC_CACHE[n1] = (_build_kernel(n1), plan_slots(n1))
    return _NC_CACHE[n1]


def prepare(feature: np.ndarray, label: np.ndarray):
    """Sort rows by label (1s first), L2-normalize; per-core input maps."""
    feature = np.ascontiguousarray(feature, dtype=np.float32)
    lab = np.asarray(label)
    perm = np.argsort(-lab, kind="stable")
    n1 = int((lab == 1).sum())
    fsort = feature[perm]
    nrm = np.sqrt((fsort.astype(np.float64) ** 2).sum(1))
    z = (fsort / np.maximum(nrm, 1e-12)[:, None].astype(np.float32)).astype(
        np.float32)
    zT = np.ascontiguousarray(z.T.astype(ml_dtypes.bfloat16))
    lsort = lab[perm].astype(np.float64)
    in_maps = []
    for c in range(N_CORES):
        tl = core_tiles(c)
        zl = np.concatenate([zT[:, 128 * T:128 * (T + 1)] for T in tl], axis=1)
        im = np.zeros((128, 2 * TILES_I), np.float32)
        for t, T in enumerate(tl):
            im[:, 2 * t] = lsort[128 * T:128 * (T + 1)]
            im[:, 2 * t + 1] = 1.0
        in_maps.append({
            "zt": zT,
            "zlt": np.ascontiguousarray(zl),
            "ind": im.astype(ml_dtypes.bfloat16),
        })
    return n1, in_maps, z, lsort


def combine(results, n1, plan, z, lsort):
    """Host-side finals from per-core partials (float64)."""
    s1 = np.zeros(B)
    s0 = np.zeros(B)
    for c, r in enumerate(results):
        P = np.asarray(r["outp"], dtype=np.float64).reshape(
            128, TILES_I, MAX_SLOTS)
        for t, T in enumerate(core_tiles(c)):
            rows = slice(128 * T, 128 * (T + 1))
            for (s, lo, hi, l1) in plan[T]:
                if l1:
                    s1[rows] += P[:, t, s]
                else:
                    s0[rows] += P[:, t, s]
        C = np.asarray(r["outc"], dtype=np.float64)  # [4 banks, 128, 512]
        C = C.reshape(4, 4, 32, GT)[:, :, 0:2, :]    # [bank, quad, 2, 512]
        cs = C.transpose(0, 1, 3, 2).reshape(B, 2)   # gtile-major -> col j
        s1 += cs[:, 0]
        s0 += cs[:, 1] - cs[:, 0]

    sall = s1 + s0
    eii = np.exp(2.0)
    same = np.where(lsort == 1.0, s1, s0)
    num = same - eii
    dennum = sall - eii
    loss = float(np.sum(np.log(dennum) - np.log(num + EPS)) / B)

    zd = z.astype(np.float64)
    S1 = (zd * lsort[:, None]).sum(0)
    S0 = zd.sum(0) - S1
    mean_pos = (S1 @ S1 + S0 @ S0 - B) / (float(B) * B)
    mean_neg = (2.0 * (S1 @ S0)) / (float(B) * B)
    return (np.float32(loss), np.float32(mean_pos), np.float32(mean_neg))


def run_on_hw(feature, label, **kwargs):
    n1, in_maps, z, lsort = prepare(feature, label)
    (nc, plan) = _get_nc(n1)
    res = run_bass_kernel_spmd(nc, in_maps,
                               core_ids=list(range(N_CORES)), **kwargs)
    return combine(res.results, n1, plan, z, lsort), res


def kernel(feature: np.ndarray, label: np.ndarray):
    out, _ = run_on_hw(feature, label)
    return out
